# revision 14
# baseline (speedup 1.0000x reference)
"""Trainium2 Bass kernel for DiffeomorphicTransform (scaling-and-squaring).

Strategy (8 NeuronCores, SPMD):
  - Output voxels sharded by z (D axis): core c owns slices [c*NZ, (c+1)*NZ).
  - Each iteration builds W2, a z-pair-interleaved copy of the full flow
    volume: W2 row r = (slab z0, y, x) holds 6 floats (slices z0, z0+1
    interleaved channel-wise).  One 12-float (48B) indirect-DMA gather then
    fetches the whole (x-pair x z-pair x 3ch) corner block at one y; two
    gathers per sample (rows y0, y0+1) cover all 8 trilinear corners.
  - Indirect DMA does 128 row-gathers per instruction (one index/partition).
  - Blend with factored tent weights on DVE/ACT.
  - AllGather (8 cores) distributes each iteration's new flow field.

Self-contained: hardcodes shapes from the problem spec.
"""
import sys

sys.path.insert(0, "/opt/trn_rl_repo")

import numpy as np

import concourse.bass as bass
import concourse.mybir as mybir
from concourse import tile

D, H, W = 160, 192, 160
TIME_STEP = 5
NCORES = 8

F32 = mybir.dt.float32
I32 = mybir.dt.int32
OP = mybir.AluOpType
COPY = mybir.ActivationFunctionType.Copy

_MAX_WAITS = 1


def split_excess_waits(nc):
    """This walrus build rejects >1 sync wait per instruction; hoist excess
    waits onto injected same-engine EventSemaphore instructions."""
    ctr = 0
    for func in nc.m.functions:
        for blk in func.blocks:
            insts = blk.instructions
            if not any(
                i.sync_info is not None
                and i.sync_info.on_wait
                and len(i.sync_info.on_wait) > _MAX_WAITS
                for i in insts
            ):
                continue
            new = []
            for inst in insts:
                si = inst.sync_info
                if si is not None and si.on_wait and len(si.on_wait) > _MAX_WAITS:
                    waits = list(si.on_wait)
                    extra, keep = waits[:-_MAX_WAITS], waits[-_MAX_WAITS:]
                    for w in extra:
                        ev = mybir.InstEventSemaphore(
                            name=f"splitw_{ctr}", ins=[], outs=[]
                        )
                        ctr += 1
                        ev.engine = inst.engine
                        ev.sync_info = mybir.SyncInfo(on_wait=[w], on_update=[])
                        new.append(ev)
                    si.on_wait = keep
                new.append(inst)
            blk.instructions = new


def build_kernel(d=D, h=H, w=W, ncores=NCORES, time_step=TIME_STEP, split=True):
    SLICE = h * w
    COLS = SLICE // 128
    assert SLICE % 128 == 0
    NZ = d // ncores
    P = d * h * w
    NSLAB = d - 1

    nc = bass.Bass(num_devices=ncores if ncores > 1 else None)

    vel = nc.dram_tensor("velocity", [P, 3], F32, kind="ExternalInput")
    vel_sh = nc.dram_tensor("vel_shard", [NZ * SLICE, 3], F32, kind="ExternalInput")
    xc_in = nc.dram_tensor("xc", [128, COLS], F32, kind="ExternalInput")
    yc_in = nc.dram_tensor("yc", [128, COLS], F32, kind="ExternalInput")
    zval_in = nc.dram_tensor("zval", [NZ, 128, COLS], F32, kind="ExternalInput")
    out_sh = nc.dram_tensor("out_shard", [NZ * SLICE, 3], F32, kind="ExternalOutput")

    inv32 = float(np.float32(1.0 / 2.0 ** time_step))
    hx = float(np.float32((w - 1) / 2.0))
    hy = float(np.float32((h - 1) / 2.0))
    hz = float(np.float32((d - 1) / 2.0))

    with tile.TileContext(nc) as tc:
        with (
            tc.tile_pool(name="dram", bufs=1, space="DRAM") as dpool,
            tc.tile_pool(name="const", bufs=1) as cpool,
            tc.tile_pool(name="bld", bufs=3) as bpool,
            tc.tile_pool(name="slab", bufs=3) as spool,
            tc.tile_pool(name="pos", bufs=2) as ppool,
            tc.tile_pool(name="gth", bufs=2) as gpool,
            tc.tile_pool(name="bl", bufs=2) as lpool,
        ):
            w2t = dpool.tile([NSLAB * SLICE, 6], F32, tag="w2", name="w2t")
            flow_full = dpool.tile([P, 3], F32, tag="ff", name="flow_full")
            mysh = [
                dpool.tile([NZ * SLICE, 3], F32, tag="msA", name="msA"),
                dpool.tile([NZ * SLICE, 3], F32, tag="msB", name="msB"),
            ]

            xc = cpool.tile([128, COLS], F32, tag="xc")
            yc = cpool.tile([128, COLS], F32, tag="yc")
            nc.sync.dma_start(out=xc[:], in_=xc_in[:])
            nc.sync.dma_start(out=yc[:], in_=yc_in[:])
            zts = []
            for s in range(NZ):
                zt = cpool.tile([128, COLS], F32, tag=f"z{s}")
                nc.sync.dma_start(out=zt[:], in_=zval_in[s])
                zts.append(zt)

            for t in range(time_step):
                src = vel[:] if t == 0 else flow_full[:]

                # ---- Phase A: build W2 (z-pair interleave) from src ----
                prev = None
                for k in range(d):
                    st = bpool.tile([128, 3 * COLS], F32, tag="bslice")
                    nc.sync.dma_start(
                        out=st[:],
                        in_=src[k * SLICE:(k + 1) * SLICE, :].rearrange(
                            "(p c) e -> p (c e)", p=128
                        ),
                    )
                    if prev is not None:
                        sl = spool.tile([128, 6 * COLS], F32, tag="bslab")
                        slv = sl[:].rearrange("p (c two e) -> p c two e", two=2, e=3)
                        nc.scalar.copy(
                            out=slv[:, :, 0, :],
                            in_=prev[:].rearrange("p (c e) -> p c e", e=3),
                        )
                        nc.scalar.copy(
                            out=slv[:, :, 1, :],
                            in_=st[:].rearrange("p (c e) -> p c e", e=3),
                        )
                        nc.sync.dma_start(
                            out=w2t[(k - 1) * SLICE:k * SLICE, :].rearrange(
                                "(p c) e -> p (c e)", p=128
                            ),
                            in_=sl[:],
                        )
                    prev = st

                # ---- Phase B: per own slice ----
                own_src = vel_sh if t == 0 else None
                for s in range(NZ):
                    f = ppool.tile([128, 3 * COLS], F32, tag="fown")
                    if t == 0:
                        nc.sync.dma_start(
                            out=f[:],
                            in_=own_src[s * SLICE:(s + 1) * SLICE, :].rearrange(
                                "(p c) e -> p (c e)", p=128
                            ),
                        )
                    else:
                        nc.sync.dma_start(
                            out=f[:],
                            in_=mysh[(t - 1) % 2][
                                s * SLICE:(s + 1) * SLICE, :
                            ].rearrange("(p c) e -> p (c e)", p=128),
                        )
                    fv = f[:].rearrange("p (c e) -> p c e", e=3)

                    def w4(tag):
                        return ppool.tile([128, COLS, 1, 1], F32, tag=tag, name=tag)

                    def v2(ap):
                        return ap.rearrange("p c a b -> p (c a b)")

                    # positions, bit-matching the reference f32 sequence:
                    #   pos = ((grid + flow) + 1) * ((L-1)/2), then clip
                    tx, ty, tz = w4("tx"), w4("ty"), w4("tz")
                    for tt_, ch, gc, half in (
                        (tx, 2, xc, hx),
                        (ty, 1, yc, hy),
                        (tz, 0, zts[s], hz),
                    ):
                        if t == 0:
                            nc.vector.tensor_scalar(
                                out=v2(tt_[:]), in0=fv[:, :, ch], scalar1=inv32,
                                scalar2=None, op0=OP.mult,
                            )
                            nc.vector.tensor_tensor(
                                out=v2(tt_[:]), in0=v2(tt_[:]), in1=gc[:], op=OP.add
                            )
                        else:
                            nc.vector.tensor_tensor(
                                out=v2(tt_[:]), in0=fv[:, :, ch], in1=gc[:], op=OP.add
                            )
                        nc.vector.tensor_scalar(
                            out=v2(tt_[:]), in0=v2(tt_[:]), scalar1=1.0,
                            scalar2=half, op0=OP.add, op1=OP.mult,
                        )
                    for tt_, L in ((tx, w), (ty, h), (tz, d)):
                        nc.vector.tensor_scalar(
                            out=v2(tt_[:]), in0=v2(tt_[:]),
                            scalar1=float(L - 1), scalar2=0.0,
                            op0=OP.min, op1=OP.max,
                        )
                    # corner base c0 = min(floor(t), L-2); upper weight a1 = t-c0
                    x0, y0, z0 = w4("x0"), w4("y0"), w4("z0")
                    a1x, a1y, a1z = w4("a1x"), w4("a1y"), w4("a1z")
                    a0x, a0y, a0z = w4("a0x"), w4("a0y"), w4("a0z")
                    for tt_, c0, a1, a0, L in (
                        (tx, x0, a1x, a0x, w),
                        (ty, y0, a1y, a0y, h),
                        (tz, z0, a1z, a0z, d),
                    ):
                        # floor via round-to-int (2^23 magic) then fix-up
                        MAGIC = float(2 ** 23)
                        nc.vector.tensor_scalar(
                            out=v2(c0[:]), in0=v2(tt_[:]), scalar1=MAGIC,
                            scalar2=MAGIC, op0=OP.add, op1=OP.subtract,
                        )  # c0 <- round(t)
                        nc.vector.tensor_tensor(
                            out=v2(a0[:]), in0=v2(c0[:]), in1=v2(tt_[:]),
                            op=OP.is_gt,
                        )  # a0 (scratch) <- 1 if round > t
                        nc.vector.tensor_tensor(
                            out=v2(c0[:]), in0=v2(c0[:]), in1=v2(a0[:]),
                            op=OP.subtract,
                        )  # c0 <- floor(t)
                        nc.vector.tensor_scalar(
                            out=v2(c0[:]), in0=v2(c0[:]), scalar1=float(L - 2),
                            scalar2=None, op0=OP.min,
                        )
                        nc.vector.tensor_tensor(
                            out=v2(a1[:]), in0=v2(tt_[:]), in1=v2(c0[:]),
                            op=OP.subtract,
                        )
                        nc.vector.tensor_scalar(
                            out=v2(a0[:]), in0=v2(a1[:]), scalar1=-1.0, scalar2=1.0,
                            op0=OP.mult, op1=OP.add,
                        )
                    # r0 = z0*SLICE + y0*W + x0 ; r1 = r0 + W
                    ra, rb = w4("ra"), w4("rb")
                    nc.vector.tensor_scalar(
                        out=v2(ra[:]), in0=v2(y0[:]), scalar1=float(w), scalar2=None,
                        op0=OP.mult,
                    )
                    nc.vector.tensor_tensor(
                        out=v2(ra[:]), in0=v2(ra[:]), in1=v2(x0[:]), op=OP.add
                    )
                    nc.vector.tensor_scalar(
                        out=v2(rb[:]), in0=v2(z0[:]), scalar1=float(SLICE),
                        scalar2=None, op0=OP.mult,
                    )
                    nc.vector.tensor_tensor(
                        out=v2(ra[:]), in0=v2(ra[:]), in1=v2(rb[:]), op=OP.add
                    )
                    r0i = ppool.tile([128, COLS], I32, tag="r0i")
                    r1i = ppool.tile([128, COLS], I32, tag="r1i")
                    nc.vector.tensor_copy(r0i[:], v2(ra[:]))
                    nc.vector.tensor_scalar(
                        out=r1i[:], in0=r0i[:], scalar1=w, scalar2=None, op0=OP.add
                    )

                    # gathers: g_dy[:, j*12:(j+1)*12] = W2[r_dy(j)] rows pair
                    g0 = gpool.tile([128, COLS * 12], F32, tag="g0")
                    g1 = gpool.tile([128, COLS * 12], F32, tag="g1")
                    for j in range(COLS):
                        nc.gpsimd.indirect_dma_start(
                            out=g0[:, j * 12:(j + 1) * 12],
                            out_offset=None,
                            in_=w2t[:],
                            in_offset=bass.IndirectOffsetOnAxis(
                                ap=r0i[:, j:j + 1], axis=0
                            ),
                        )
                        nc.gpsimd.indirect_dma_start(
                            out=g1[:, j * 12:(j + 1) * 12],
                            out_offset=None,
                            in_=w2t[:],
                            in_offset=bass.IndirectOffsetOnAxis(
                                ap=r1i[:, j:j + 1], axis=0
                            ),
                        )

                    # blend, replicating the reference's product/sum tree:
                    #   w = (wx*wy)*wz ; out = (((c000+c001)+c010)+...)  in
                    #   (dz, dy, dx) loop order, dx fastest.
                    # g layout per j: (dx2, dz2, c3); g0=row y0, g1=row y0+1
                    g0v = g0[:].rearrange("p (c dx dz e) -> p c dx dz e", dx=2, dz=2, e=3)
                    g1v = g1[:].rearrange("p (c dx dz e) -> p c dx dz e", dx=2, dz=2, e=3)
                    wxy = {}
                    for dy, ay in ((0, a0y), (1, a1y)):
                        for dx, ax in ((0, a0x), (1, a1x)):
                            m = ppool.tile(
                                [128, COLS, 1, 1], F32, tag=f"wxy{dy}{dx}",
                                name=f"wxy{dy}{dx}",
                            )
                            nc.vector.tensor_tensor(
                                out=v2(m[:]), in0=v2(ax[:]), in1=v2(ay[:]),
                                op=OP.mult,
                            )
                            wxy[(dy, dx)] = m
                    V1 = lpool.tile([128, COLS, 3], F32, tag="V1")
                    V2 = lpool.tile([128, COLS, 3], F32, tag="V2")
                    wc = lpool.tile([128, COLS, 1, 1], F32, tag="wc")
                    first = True
                    for dz, az in ((0, a0z), (1, a1z)):
                        for dy in (0, 1):
                            gsel = g0v if dy == 0 else g1v
                            for dx in (0, 1):
                                nc.vector.tensor_tensor(
                                    out=v2(wc[:]), in0=v2(wxy[(dy, dx)][:]),
                                    in1=v2(az[:]), op=OP.mult,
                                )
                                wcb = wc[:].rearrange(
                                    "p c a b -> p c (a b)"
                                ).to_broadcast([128, COLS, 3])
                                dst = V1 if first else V2
                                nc.vector.tensor_tensor(
                                    out=dst[:], in0=gsel[:, :, dx, dz, :], in1=wcb,
                                    op=OP.mult,
                                )
                                if not first:
                                    nc.vector.tensor_tensor(
                                        out=V1[:], in0=V1[:], in1=V2[:], op=OP.add
                                    )
                                first = False
                    # add own flow (and global 1/32 scale on iteration 0)
                    nc.vector.tensor_tensor(out=V1[:], in0=V1[:], in1=fv, op=OP.add)
                    if t == 0:
                        nc.vector.tensor_scalar(
                            out=V1[:], in0=V1[:], scalar1=inv32, scalar2=None,
                            op0=OP.mult,
                        )
                    newv = V1[:].rearrange("p c e -> p (c e)")
                    if t == time_step - 1:
                        nc.sync.dma_start(
                            out=out_sh[s * SLICE:(s + 1) * SLICE, :].rearrange(
                                "(p c) e -> p (c e)", p=128
                            ),
                            in_=newv,
                        )
                    else:
                        nc.sync.dma_start(
                            out=mysh[t % 2][s * SLICE:(s + 1) * SLICE, :].rearrange(
                                "(p c) e -> p (c e)", p=128
                            ),
                            in_=newv,
                        )

                # ---- Phase C: all-gather new flow ----
                if t < time_step - 1:
                    if ncores > 1:
                        nc.gpsimd.collective_compute(
                            "AllGather",
                            OP.bypass,
                            replica_groups=[list(range(ncores))],
                            ins=[mysh[t % 2][:]],
                            outs=[flow_full[:]],
                        )
                    else:
                        nc.sync.dma_start(out=flow_full[:], in_=mysh[t % 2][:])

    if split:
        split_excess_waits(nc)
    return nc


def _grid_axis(n):
    # bit-identical to the reference's f32 sequence:
    #   (arange(n) - (n-1)/2) / (n-1) * 2
    a = np.arange(n, dtype=np.float32)
    return (a - np.float32((n - 1) / 2)) / np.float32(n - 1) * np.float32(2)


def make_const_inputs(core, d=D, h=H, w=W, ncores=NCORES):
    SLICE = h * w
    COLS = SLICE // 128
    NZ = d // ncores
    gx, gy, gz = _grid_axis(w), _grid_axis(h), _grid_axis(d)
    v = np.arange(SLICE, dtype=np.int64).reshape(128, COLS)
    xc = gx[v % w].astype(np.float32)
    yc = gy[v // w].astype(np.float32)
    zval = np.zeros((NZ, 128, COLS), np.float32)
    for s in range(NZ):
        zval[s] = gz[core * NZ + s]
    return xc, yc, zval


_CACHE = {}


def _get_runner():
    if "r" in _CACHE:
        return _CACHE["r"]
    import jax
    from jax.sharding import Mesh, PartitionSpec, NamedSharding
    from jax.experimental.shard_map import shard_map
    from concourse.bass2jax import (
        _bass_exec_p,
        install_neuronx_cc_hook,
        partition_id_tensor,
    )

    nc = build_kernel()
    install_neuronx_cc_hook()
    partition_name = nc.partition_id_tensor.name if nc.partition_id_tensor else None
    in_names, out_names, out_avals, zero_outs = [], [], [], []
    for alloc in nc.m.functions[0].allocations:
        if not isinstance(alloc, mybir.MemoryLocationSet):
            continue
        name = alloc.memorylocations[0].name
        if alloc.kind == "ExternalInput":
            if name != partition_name:
                in_names.append(name)
        elif alloc.kind == "ExternalOutput":
            shape = tuple(alloc.tensor_shape)
            dt = mybir.dt.np(alloc.dtype)
            out_names.append(name)
            out_avals.append(jax.core.ShapedArray(shape, dt))
            zero_outs.append(np.zeros(shape, dt))
    all_in_names = in_names + out_names + ([partition_name] if partition_name else [])

    def _body(*args):
        operands = list(args)
        if partition_name is not None:
            operands.append(partition_id_tensor())
        outs = _bass_exec_p.bind(
            *operands,
            out_avals=tuple(out_avals),
            in_names=tuple(all_in_names),
            out_names=tuple(out_names),
            lowering_input_output_aliases=(),
            sim_require_finite=False,
            sim_require_nnan=False,
            nc=nc,
        )
        return tuple(outs)

    devices = jax.devices()[:NCORES]
    mesh = Mesh(np.asarray(devices), ("core",))
    n_params = len(in_names)
    n_outs = len(out_avals)
    in_specs = (PartitionSpec("core"),) * (n_params + n_outs)
    out_specs = (PartitionSpec("core"),) * n_outs
    fn = jax.jit(
        shard_map(
            _body, mesh=mesh, in_specs=in_specs, out_specs=out_specs, check_rep=False
        ),
        keep_unused=True,
    )
    _CACHE["r"] = (fn, mesh, in_names, out_names, out_avals, zero_outs)
    return _CACHE["r"]


def kernel(velocity):
    import jax

    velocity = np.ascontiguousarray(velocity, dtype=np.float32)
    fn, mesh, in_names, out_names, out_avals, zero_outs = _get_runner()
    SLICE = H * W
    NZ = D // NCORES
    in_maps = []
    for c in range(NCORES):
        xc, yc, zval = make_const_inputs(c)
        in_maps.append(
            {
                "velocity": velocity,
                "vel_shard": velocity[c * NZ * SLICE:(c + 1) * NZ * SLICE],
                "xc": xc,
                "yc": yc,
                "zval": zval,
            }
        )
    args = []
    for name in in_names:
        glob = np.concatenate([m[name] for m in in_maps], axis=0)
        args.append(glob)
    for z in zero_outs:
        args.append(np.zeros((NCORES * z.shape[0], *z.shape[1:]), z.dtype))
    outs = fn(*args)
    jax.block_until_ready(outs)
    i = out_names.index("out_shard")
    res = np.asarray(outs[i])  # [NCORES*NZ*SLICE, 3] concat by core = z order
    return res.reshape(-1, 3)
